# revision 1
# baseline (speedup 1.0000x reference)
# Janossy pooling (K=2) Trainium2 kernel.
#
# Reference computation:
#   perms = all ordered pairs (i, j), i != j, of N=32 set elements (P=992)
#   h1 = relu(concat(x_i, x_j) @ W1 + b1)          [B, P, 512]
#   h2 = relu(h1 @ W2 + b2)                        [B, P, 512]
#   out = sum_p (h2 @ W3 + b3)                     [B, 256]
#
# Algebraic restructuring used here:
#   * Layer 1 factorizes: concat(x_i, x_j) @ W1 = x_i @ W1[:128] + x_j @ W1[128:]
#     so per-element projections A = x@W1a + b1, C = x@W1b are computed once
#     (B*N rows instead of B*P).
#   * Layer 3 commutes with the pooling sum:
#     sum_p (h2_p @ W3 + b3) = (sum_p h2_p) @ W3 + P*b3,
#     so only the pooled h2 sum (per set) is needed - h2 is never materialized.
#   * The ordered-pair sum over i != j is computed as the full N x N grid sum
#     minus the diagonal (i == j) terms, which are computed separately (N rows
#     per set instead of N^2) and folded with negative sign into the final
#     W3 accumulation.
#
# Layout: everything flows transposed ([feature, row]) so that
#   - pre1(i,j) = A_i + C_j is a single broadcast tensor_tensor per block,
#   - b2 is a per-partition activation bias,
#   - the pooling sum is the activation's fused accum_out (free-dim sum),
#   - the pooled vectors land directly in the lhsT layout the W3 matmul needs.
# Matmuls run as float32r (fp22 multiply, fp32 accumulate): 4x the fp32 rate.
#
# Sharding: data-parallel over the batch dim, 8 sets per core, weights
# replicated; each core computes its 8 output rows independently (no
# collectives), gathered host-side.

import numpy as np

B, N, IN_DIM, H_DIM, OUT_DIM = 64, 32, 128, 512, 256
P_PERM = N * (N - 1)  # 992
N_CORES = 8
SETS_PER_CORE = B // N_CORES  # 8
TOK = SETS_PER_CORE * N  # 256 tokens (set elements) per core
N_BLOCKS = SETS_PER_CORE * 2  # 16 blocks of 512 (i,j) rows each
I_PER_BLK = 16  # i values per block (x 32 j values = 512 rows)

_cache = {}


def _build_nc():
    import concourse.bacc as bacc
    import concourse.mybir as mybir
    import concourse.tile as tile

    f32 = mybir.dt.float32
    f32r = mybir.dt.float32r
    Alu = mybir.AluOpType
    Act = mybir.ActivationFunctionType

    nc = bacc.Bacc("TRN2", target_bir_lowering=False, debug=False)

    # ---- DRAM I/O (per core; x differs per core, the rest replicated) ----
    x_d = nc.dram_tensor("x", [TOK, IN_DIM], f32, kind="ExternalInput")
    W1_d = nc.dram_tensor("W1", [2 * IN_DIM, H_DIM], f32, kind="ExternalInput")
    W2_d = nc.dram_tensor("W2", [H_DIM, H_DIM], f32, kind="ExternalInput")
    W3_d = nc.dram_tensor("W3", [H_DIM, OUT_DIM], f32, kind="ExternalInput")
    b1T_d = nc.dram_tensor("b1T", [128, 4], f32, kind="ExternalInput")
    b2T_d = nc.dram_tensor("b2T", [128, 4], f32, kind="ExternalInput")
    b3r_d = nc.dram_tensor("b3r", [1, OUT_DIM], f32, kind="ExternalInput")
    eye_d = nc.dram_tensor("eye", [128, 128], f32, kind="ExternalInput")
    ones8_d = nc.dram_tensor("ones8", [1, 8], f32, kind="ExternalInput")
    y_d = nc.dram_tensor("y", [SETS_PER_CORE, OUT_DIM], f32, kind="ExternalOutput")

    with tile.TileContext(nc) as tc:
        with (
            tc.tile_pool(name="const", bufs=1) as cpool,
            tc.tile_pool(name="pre", bufs=2) as prepool,
            tc.tile_pool(name="h1", bufs=2) as h1pool,
            tc.tile_pool(name="scr", bufs=3) as scrpool,
            tc.tile_pool(name="ps", bufs=2, space="PSUM") as pspool,
        ):
            # ---- constants into SBUF ----
            xsb = cpool.tile([128, 2, 128], f32)
            nc.sync.dma_start(xsb[:], x_d[:].rearrange("(c p) f -> p c f", p=128))
            eye = cpool.tile([128, 128], f32)
            nc.sync.dma_start(eye[:], eye_d[:])
            W1a = cpool.tile([128, H_DIM], f32r)
            nc.sync.dma_start(W1a[:], W1_d[0:128, :].bitcast(f32r))
            W1b = cpool.tile([128, H_DIM], f32r)
            nc.sync.dma_start(W1b[:], W1_d[128:256, :].bitcast(f32r))
            W2sb = cpool.tile([128, 4, H_DIM], f32r)
            nc.sync.dma_start(
                W2sb[:], W2_d[:].rearrange("(c p) h -> p c h", p=128).bitcast(f32r)
            )
            W3sb = cpool.tile([128, 4, OUT_DIM], f32r)
            nc.sync.dma_start(
                W3sb[:], W3_d[:].rearrange("(c p) o -> p c o", p=128).bitcast(f32r)
            )
            b1T = cpool.tile([128, 4], f32)
            nc.sync.dma_start(b1T[:], b1T_d[:])
            b2T = cpool.tile([128, 4], f32)
            nc.sync.dma_start(b2T[:], b2T_d[:])
            b3r = cpool.tile([1, OUT_DIM], f32r)
            nc.sync.dma_start(b3r[:], b3r_d[:].bitcast(f32r))
            ones8 = cpool.tile([1, 8], f32r)
            nc.sync.dma_start(ones8[:], ones8_d[:].bitcast(f32r))
            zero1 = cpool.tile([128, 1], f32)
            nc.vector.memset(zero1[:], 0.0)

            # persistent accumulators / staging
            xT = cpool.tile([128, TOK], f32r)  # x transposed [feat, token]
            A_T = cpool.tile([128, 4, TOK], f32)  # (x@W1a + b1) transposed
            C_T = cpool.tile([128, 4, TOK], f32)  # (x@W1b) transposed
            pool_sb = cpool.tile([128, 4, N_BLOCKS], f32)  # pooled h2 sums
            pool_r = cpool.tile([128, 4, N_BLOCKS], f32r)
            h2dT = cpool.tile([128, 4, TOK], f32)  # diagonal h2
            dST = cpool.tile([128, 4, 8], f32)  # per-set diag sums
            dST_r = cpool.tile([128, 4, 8], f32r)
            y_sb = cpool.tile([8, OUT_DIM], f32)

            # ---- preprocessing: transpose x, project to A_T / C_T ----
            for c in range(2):
                ps = pspool.tile([128, 4, 512], f32, tag="ps")
                nc.tensor.transpose(ps[:, 0, 0:128], xsb[:, c, :], eye[:])
                nc.vector.tensor_copy(
                    xT[:, c * 128 : (c + 1) * 128], ps[:, 0, 0:128].bitcast(f32r)
                )
            for hc in range(4):
                ps = pspool.tile([128, 4, 512], f32, tag="ps")
                nc.tensor.matmul(
                    ps[:, 0, 0:TOK], W1a[:, hc * 128 : (hc + 1) * 128], xT[:]
                )
                nc.scalar.add(A_T[:, hc, :], ps[:, 0, 0:TOK], b1T[:, hc : hc + 1])
            for hc in range(4):
                ps = pspool.tile([128, 4, 512], f32, tag="ps")
                nc.tensor.matmul(
                    ps[:, 0, 0:TOK], W1b[:, hc * 128 : (hc + 1) * 128], xT[:]
                )
                nc.vector.tensor_copy(C_T[:, hc, :], ps[:, 0, 0:TOK])

            # ---- diagonal terms (i == j): computed separately, negated ----
            pre1d = prepool.tile([128, 4, TOK], f32, tag="pre1d")
            nc.vector.tensor_tensor(
                pre1d[:].rearrange("p c t -> p (c t)"),
                A_T[:].rearrange("p c t -> p (c t)"),
                C_T[:].rearrange("p c t -> p (c t)"),
                Alu.add,
            )
            h1dT = h1pool.tile([128, 4, TOK], f32r, tag="h1d")
            nc.vector.tensor_scalar(
                h1dT[:].rearrange("p c t -> p (c t)"),
                pre1d[:].rearrange("p c t -> p (c t)"),
                0.0,
                None,
                Alu.max,
            )
            psd = pspool.tile([128, 4, 512], f32, tag="ps")
            for f in range(4):
                for kc in range(4):
                    nc.tensor.matmul(
                        psd[:, f, 0:TOK],
                        W2sb[:, kc, f * 128 : (f + 1) * 128],
                        h1dT[:, kc, :],
                        start=(kc == 0),
                        stop=(kc == 3),
                    )
            for f in range(4):
                nc.scalar.activation(
                    h2dT[:, f, :], psd[:, f, 0:TOK], Act.Relu, bias=b2T[:, f : f + 1]
                )
            for f in range(4):
                for s in range(8):
                    nc.vector.tensor_reduce(
                        dST[:, f, s : s + 1],
                        h2dT[:, f, s * N : (s + 1) * N],
                        mybir.AxisListType.X,
                        Alu.add,
                    )
            nc.vector.tensor_scalar_mul(
                dST_r[:].rearrange("p c s -> p (c s)"),
                dST[:].rearrange("p c s -> p (c s)"),
                -1.0,
            )

            # ---- main grid: 16 blocks of 512 (i, j) rows ----
            for blk in range(N_BLOCKS):
                b, h = blk // 2, blk % 2
                t0 = b * N + h * I_PER_BLK
                pre1 = prepool.tile([128, 4, I_PER_BLK, N], f32, tag="pre1")
                nc.vector.tensor_tensor(
                    pre1[:],
                    A_T[:, :, t0 : t0 + I_PER_BLK]
                    .unsqueeze(3)
                    .broadcast_to([128, 4, I_PER_BLK, N]),
                    C_T[:, :, b * N : (b + 1) * N]
                    .unsqueeze(2)
                    .broadcast_to([128, 4, I_PER_BLK, N]),
                    Alu.add,
                )
                h1T = h1pool.tile([128, 4, 512], f32r, tag="h1")
                pre_f = pre1[:].rearrange("p c i j -> p (c i j)")
                h1_f = h1T[:].rearrange("p c m -> p (c m)")
                nc.vector.tensor_scalar(
                    h1_f[:, 0:512], pre_f[:, 0:512], 0.0, None, Alu.max
                )
                nc.scalar.activation(h1_f[:, 512:2048], pre_f[:, 512:2048], Act.Relu)

                ps = pspool.tile([128, 4, 512], f32, tag="ps")
                for f in range(4):
                    for kc in range(4):
                        nc.tensor.matmul(
                            ps[:, f, :],
                            W2sb[:, kc, f * 128 : (f + 1) * 128],
                            h1T[:, kc, :],
                            start=(kc == 0),
                            stop=(kc == 3),
                        )
                # evacuate: relu(pre2 + b2) and pooled (free-dim) sum in one op
                for f in range(3):
                    scr = scrpool.tile([128, 512], f32, tag="scr")
                    nc.scalar.activation(
                        scr[:],
                        ps[:, f, :],
                        Act.Relu,
                        bias=b2T[:, f : f + 1],
                        accum_out=pool_sb[:, f, blk : blk + 1],
                    )
                scr = scrpool.tile([128, 512], f32, tag="scr")
                nc.vector.scalar_tensor_tensor(
                    scr[:],
                    ps[:, 3, :],
                    b2T[:, 3:4],
                    zero1[:].broadcast_to([128, 512]),
                    Alu.add,
                    Alu.max,
                    accum_out=pool_sb[:, 3, blk : blk + 1],
                )

            # ---- tail: out = (grid_sum - diag_sum) @ W3 + P*b3 ----
            nc.vector.tensor_copy(
                pool_r[:].rearrange("p c m -> p (c m)"),
                pool_sb[:].rearrange("p c m -> p (c m)"),
            )
            out_ps = pspool.tile([128, 4, 512], f32, tag="ps")
            out_v = out_ps[0:8, 0, 0:OUT_DIM]
            first = True
            for gc in range(4):
                for h in range(2):
                    nc.tensor.matmul(
                        out_v,
                        pool_r[:, gc, h : N_BLOCKS : 2],
                        W3sb[:, gc, :],
                        start=first,
                        stop=False,
                    )
                    first = False
            for gc in range(4):
                nc.tensor.matmul(
                    out_v, dST_r[:, gc, :], W3sb[:, gc, :], start=False, stop=False
                )
            nc.tensor.matmul(out_v, ones8[:], b3r[:], start=False, stop=True)
            nc.vector.tensor_copy(y_sb[:], out_v)
            nc.sync.dma_start(y_d[:], y_sb[:])

    nc.compile()
    return nc


def _prep_shared(W1, b1, W2, b2, W3, b3):
    return {
        "W1": np.ascontiguousarray(W1, dtype=np.float32),
        "W2": np.ascontiguousarray(W2, dtype=np.float32),
        "W3": np.ascontiguousarray(W3, dtype=np.float32),
        "b1T": np.ascontiguousarray(b1.reshape(4, 128).T, dtype=np.float32),
        "b2T": np.ascontiguousarray(b2.reshape(4, 128).T, dtype=np.float32),
        "b3r": (P_PERM * np.asarray(b3, dtype=np.float32)).reshape(1, OUT_DIM),
        "eye": np.eye(128, dtype=np.float32),
        "ones8": np.ones((1, 8), dtype=np.float32),
    }


def kernel(x, W1, b1, W2, b2, W3, b3, _trace=False):
    from concourse.bass_utils import run_bass_kernel_spmd

    if "nc" not in _cache:
        _cache["nc"] = _build_nc()
    nc = _cache["nc"]

    shared = _prep_shared(W1, b1, W2, b2, W3, b3)
    x = np.ascontiguousarray(x, dtype=np.float32)
    in_maps = []
    for c in range(N_CORES):
        m = dict(shared)
        m["x"] = np.ascontiguousarray(
            x[c * SETS_PER_CORE : (c + 1) * SETS_PER_CORE].reshape(TOK, IN_DIM)
        )
        in_maps.append(m)

    res = run_bass_kernel_spmd(
        nc, in_maps, core_ids=list(range(N_CORES)), trace=_trace
    )
    y = np.concatenate([res.results[c]["y"] for c in range(N_CORES)], axis=0)
    if _trace:
        kernel.last_result = res
    return y.astype(np.float32)


# revision 4
# speedup vs baseline: 1.0985x; 1.0985x over previous
# Janossy pooling (K=2) Trainium2 kernel.
#
# Reference computation:
#   perms = all ordered pairs (i, j), i != j, of N=32 set elements (P=992)
#   h1 = relu(concat(x_i, x_j) @ W1 + b1)          [B, P, 512]
#   h2 = relu(h1 @ W2 + b2)                        [B, P, 512]
#   out = sum_p (h2 @ W3 + b3)                     [B, 256]
#
# Algebraic restructuring used here:
#   * Layer 1 factorizes: concat(x_i, x_j) @ W1 = x_i @ W1[:128] + x_j @ W1[128:]
#     so per-element projections A = x@W1a + b1, C = x@W1b are computed once
#     (B*N rows instead of B*P).
#   * Layer 3 commutes with the pooling sum:
#     sum_p (h2_p @ W3 + b3) = (sum_p h2_p) @ W3 + P*b3,
#     so only the pooled h2 sum (per set) is needed - h2 is never materialized.
#   * The ordered-pair sum over i != j is computed as the full N x N grid sum
#     minus the diagonal (i == j) terms, which are computed separately (N rows
#     per set instead of N^2) and folded with negative sign into the final
#     W3 accumulation.
#
# Layout: everything flows transposed ([feature, row]) so that
#   - pre1(i,j) = A_i + C_j is a single broadcast tensor_tensor per block,
#   - b2 is a per-partition activation bias,
#   - the pooling sum is the activation's fused accum_out (free-dim sum),
#   - the pooled vectors land directly in the lhsT layout the W3 matmul needs.
# Matmuls run as float32r (fp22 multiply, fp32 accumulate): 4x the fp32 rate.
#
# Sharding: data-parallel over the batch dim, 8 sets per core, weights
# replicated; each core computes its 8 output rows independently (no
# collectives), gathered host-side.

import numpy as np

B, N, IN_DIM, H_DIM, OUT_DIM = 64, 32, 128, 512, 256
P_PERM = N * (N - 1)  # 992
N_CORES = 8
SETS_PER_CORE = B // N_CORES  # 8
TOK = SETS_PER_CORE * N  # 256 tokens (set elements) per core
N_BLOCKS = SETS_PER_CORE * 2  # 16 blocks of 512 (i,j) rows each
I_PER_BLK = 16  # i values per block (x 32 j values = 512 rows)

_cache = {}


def _build_nc():
    import concourse.bacc as bacc
    import concourse.mybir as mybir
    import concourse.tile as tile

    f32 = mybir.dt.float32
    f32r = mybir.dt.float32r
    bf16 = mybir.dt.bfloat16
    Alu = mybir.AluOpType
    Act = mybir.ActivationFunctionType

    nc = bacc.Bacc("TRN2", target_bir_lowering=False, debug=False)

    # ---- DRAM I/O (per core; x differs per core, the rest replicated) ----
    x_d = nc.dram_tensor("x", [TOK, IN_DIM], f32, kind="ExternalInput")
    W1_d = nc.dram_tensor("W1", [2 * IN_DIM, H_DIM], f32, kind="ExternalInput")
    W2_d = nc.dram_tensor("W2", [H_DIM, H_DIM], bf16, kind="ExternalInput")
    W3_d = nc.dram_tensor("W3", [H_DIM, OUT_DIM], f32, kind="ExternalInput")
    b1T_d = nc.dram_tensor("b1T", [128, 4], f32, kind="ExternalInput")
    b2T_d = nc.dram_tensor("b2T", [128, 4], f32, kind="ExternalInput")
    b3r_d = nc.dram_tensor("b3r", [1, OUT_DIM], f32, kind="ExternalInput")
    eye_d = nc.dram_tensor("eye", [128, 128], f32, kind="ExternalInput")
    ones8_d = nc.dram_tensor("ones8", [1, 8], f32, kind="ExternalInput")
    y_d = nc.dram_tensor("y", [SETS_PER_CORE, OUT_DIM], f32, kind="ExternalOutput")

    with tile.TileContext(nc) as tc:
        with (
            tc.tile_pool(name="const", bufs=1) as cpool,
            tc.tile_pool(name="pre", bufs=2) as prepool,
            tc.tile_pool(name="h1", bufs=2) as h1pool,
            tc.tile_pool(name="scr", bufs=3) as scrpool,
            tc.tile_pool(name="ps", bufs=2, space="PSUM") as pspool,
        ):
            # ---- constants into SBUF ----
            xsb = cpool.tile([128, 2, 128], f32)
            nc.sync.dma_start(xsb[:], x_d[:].rearrange("(c p) f -> p c f", p=128))
            eye = cpool.tile([128, 128], f32)
            nc.sync.dma_start(eye[:], eye_d[:])
            W1a = cpool.tile([128, H_DIM], f32r)
            nc.sync.dma_start(W1a[:], W1_d[0:128, :].bitcast(f32r))
            W1b = cpool.tile([128, H_DIM], f32r)
            nc.sync.dma_start(W1b[:], W1_d[128:256, :].bitcast(f32r))
            W2sb = cpool.tile([128, 4, H_DIM], bf16)
            nc.sync.dma_start(
                W2sb[:], W2_d[:].rearrange("(c p) h -> p c h", p=128)
            )
            W3sb = cpool.tile([128, 4, OUT_DIM], f32r)
            nc.sync.dma_start(
                W3sb[:], W3_d[:].rearrange("(c p) o -> p c o", p=128).bitcast(f32r)
            )
            b1T = cpool.tile([128, 4], f32)
            nc.sync.dma_start(b1T[:], b1T_d[:])
            b2T = cpool.tile([128, 4], f32)
            nc.sync.dma_start(b2T[:], b2T_d[:])
            b3r = cpool.tile([1, OUT_DIM], f32r)
            nc.sync.dma_start(b3r[:], b3r_d[:].bitcast(f32r))
            ones8 = cpool.tile([1, 8], f32r)
            nc.sync.dma_start(ones8[:], ones8_d[:].bitcast(f32r))
            zero1 = cpool.tile([128, 1], f32)
            nc.vector.memset(zero1[:], 0.0)

            # persistent accumulators / staging
            xT = cpool.tile([128, TOK], f32r)  # x transposed [feat, token]
            A_T = cpool.tile([128, 4, TOK], f32)  # (x@W1a + b1) transposed
            C_T = cpool.tile([128, 4, TOK], f32)  # (x@W1b) transposed
            pool_sb = cpool.tile([128, 4, N_BLOCKS], f32)  # pooled h2 sums
            pool_r = cpool.tile([128, 4, N_BLOCKS], f32r)
            h2dT = cpool.tile([128, 4, TOK], f32)  # diagonal h2
            dST = cpool.tile([128, 4, 8], f32)  # per-set diag sums
            dST_r = cpool.tile([128, 4, 8], f32r)
            y_sb = cpool.tile([8, OUT_DIM], f32)

            # ---- preprocessing: transpose x, project to A_T / C_T ----
            for c in range(2):
                ps = pspool.tile([128, 4, 512], f32, tag="ps")
                nc.tensor.transpose(ps[:, 0, 0:128], xsb[:, c, :], eye[:])
                nc.vector.tensor_copy(
                    xT[:, c * 128 : (c + 1) * 128], ps[:, 0, 0:128].bitcast(f32r)
                )
            for hc in range(4):
                ps = pspool.tile([128, 4, 512], f32, tag="ps")
                nc.tensor.matmul(
                    ps[:, 0, 0:TOK], W1a[:, hc * 128 : (hc + 1) * 128], xT[:]
                )
                nc.scalar.add(A_T[:, hc, :], ps[:, 0, 0:TOK], b1T[:, hc : hc + 1])
            for hc in range(4):
                ps = pspool.tile([128, 4, 512], f32, tag="ps")
                nc.tensor.matmul(
                    ps[:, 0, 0:TOK], W1b[:, hc * 128 : (hc + 1) * 128], xT[:]
                )
                nc.vector.tensor_copy(C_T[:, hc, :], ps[:, 0, 0:TOK])

            # ---- diagonal terms (i == j): computed separately, negated ----
            pre1d = prepool.tile([128, 4, TOK], f32, tag="pre1d")
            nc.vector.tensor_tensor(
                pre1d[:].rearrange("p c t -> p (c t)"),
                A_T[:].rearrange("p c t -> p (c t)"),
                C_T[:].rearrange("p c t -> p (c t)"),
                Alu.add,
            )
            h1dT = h1pool.tile([128, 4, TOK], bf16, tag="h1d")
            nc.vector.tensor_scalar(
                h1dT[:].rearrange("p c t -> p (c t)"),
                pre1d[:].rearrange("p c t -> p (c t)"),
                0.0,
                None,
                Alu.max,
            )
            psd = pspool.tile([128, 4, 512], f32, tag="ps")
            for f in range(4):
                for kc in range(4):
                    nc.tensor.matmul(
                        psd[:, f, 0:TOK],
                        W2sb[:, kc, f * 128 : (f + 1) * 128],
                        h1dT[:, kc, :],
                        start=(kc == 0),
                        stop=(kc == 3),
                    )
            for f in range(4):
                nc.scalar.activation(
                    h2dT[:, f, :], psd[:, f, 0:TOK], Act.Relu, bias=b2T[:, f : f + 1]
                )
            for f in range(4):
                for s in range(8):
                    nc.vector.tensor_reduce(
                        dST[:, f, s : s + 1],
                        h2dT[:, f, s * N : (s + 1) * N],
                        mybir.AxisListType.X,
                        Alu.add,
                    )
            nc.vector.tensor_scalar_mul(
                dST_r[:].rearrange("p c s -> p (c s)"),
                dST[:].rearrange("p c s -> p (c s)"),
                -1.0,
            )

            # ---- main grid: 16 blocks of 512 (i, j) rows ----
            for blk in range(N_BLOCKS):
                b, h = blk // 2, blk % 2
                t0 = b * N + h * I_PER_BLK
                pre1 = prepool.tile([128, 4, I_PER_BLK, N], f32, tag="pre1")
                nc.vector.tensor_tensor(
                    pre1[:],
                    A_T[:, :, t0 : t0 + I_PER_BLK]
                    .unsqueeze(3)
                    .broadcast_to([128, 4, I_PER_BLK, N]),
                    C_T[:, :, b * N : (b + 1) * N]
                    .unsqueeze(2)
                    .broadcast_to([128, 4, I_PER_BLK, N]),
                    Alu.add,
                )
                h1T = h1pool.tile([128, 4, 512], bf16, tag="h1")
                pre_f = pre1[:].rearrange("p c i j -> p (c i j)")
                h1_f = h1T[:].rearrange("p c m -> p (c m)")
                nc.vector.tensor_scalar(
                    h1_f[:, 0:512], pre_f[:, 0:512], 0.0, None, Alu.max
                )
                nc.scalar.activation(h1_f[:, 512:2048], pre_f[:, 512:2048], Act.Relu)

                ps = pspool.tile([128, 4, 512], f32, tag="ps")
                for f in range(4):
                    for kc in range(4):
                        nc.tensor.matmul(
                            ps[:, f, :],
                            W2sb[:, kc, f * 128 : (f + 1) * 128],
                            h1T[:, kc, :],
                            start=(kc == 0),
                            stop=(kc == 3),
                        )
                # evacuate: relu(pre2 + b2) and pooled (free-dim) sum in one op
                for f in range(3):
                    scr = scrpool.tile([128, 512], f32, tag="scr")
                    nc.scalar.activation(
                        scr[:],
                        ps[:, f, :],
                        Act.Relu,
                        bias=b2T[:, f : f + 1],
                        accum_out=pool_sb[:, f, blk : blk + 1],
                    )
                scr = scrpool.tile([128, 512], f32, tag="scr")
                nc.vector.scalar_tensor_tensor(
                    scr[:],
                    ps[:, 3, :],
                    b2T[:, 3:4],
                    zero1[:].broadcast_to([128, 512]),
                    Alu.add,
                    Alu.max,
                    accum_out=pool_sb[:, 3, blk : blk + 1],
                )

            # ---- tail: out = (grid_sum - diag_sum) @ W3 + P*b3 ----
            nc.vector.tensor_copy(
                pool_r[:].rearrange("p c m -> p (c m)"),
                pool_sb[:].rearrange("p c m -> p (c m)"),
            )
            out_ps = pspool.tile([128, 4, 512], f32, tag="ps")
            out_v = out_ps[0:8, 0, 0:OUT_DIM]
            first = True
            for gc in range(4):
                for h in range(2):
                    nc.tensor.matmul(
                        out_v,
                        pool_r[:, gc, h : N_BLOCKS : 2],
                        W3sb[:, gc, :],
                        start=first,
                        stop=False,
                    )
                    first = False
            for gc in range(4):
                nc.tensor.matmul(
                    out_v, dST_r[:, gc, :], W3sb[:, gc, :], start=False, stop=False
                )
            nc.tensor.matmul(out_v, ones8[:], b3r[:], start=False, stop=True)
            nc.vector.tensor_copy(y_sb[:], out_v)
            nc.sync.dma_start(y_d[:], y_sb[:])

    nc.compile()
    return nc


def _bf16_dt():
    import ml_dtypes

    return ml_dtypes.bfloat16


def _prep_shared(W1, b1, W2, b2, W3, b3):
    return {
        "W1": np.ascontiguousarray(W1, dtype=np.float32),
        "W2": np.ascontiguousarray(
            np.asarray(W2, dtype=np.float32), dtype=_bf16_dt()
        ),
        "W3": np.ascontiguousarray(W3, dtype=np.float32),
        "b1T": np.ascontiguousarray(b1.reshape(4, 128).T, dtype=np.float32),
        "b2T": np.ascontiguousarray(b2.reshape(4, 128).T, dtype=np.float32),
        "b3r": (P_PERM * np.asarray(b3, dtype=np.float32)).reshape(1, OUT_DIM),
        "eye": np.eye(128, dtype=np.float32),
        "ones8": np.ones((1, 8), dtype=np.float32),
    }


def kernel(x, W1, b1, W2, b2, W3, b3, _trace=False):
    from concourse.bass_utils import run_bass_kernel_spmd

    if "nc" not in _cache:
        _cache["nc"] = _build_nc()
    nc = _cache["nc"]

    shared = _prep_shared(W1, b1, W2, b2, W3, b3)
    x = np.ascontiguousarray(x, dtype=np.float32)
    in_maps = []
    for c in range(N_CORES):
        m = dict(shared)
        m["x"] = np.ascontiguousarray(
            x[c * SETS_PER_CORE : (c + 1) * SETS_PER_CORE].reshape(TOK, IN_DIM)
        )
        in_maps.append(m)

    res = run_bass_kernel_spmd(
        nc, in_maps, core_ids=list(range(N_CORES)), trace=_trace
    )
    y = np.concatenate([res.results[c]["y"] for c in range(N_CORES)], axis=0)
    if _trace:
        kernel.last_result = res
    return y.astype(np.float32)
